# revision 24
# baseline (speedup 1.0000x reference)
"""Trainium2 Bass kernel for nn_CrossAttentionReranker (feature-major).

Math folding (seq_len==1 everywhere, ln_w==1/ln_b==0, all biases==0 --
asserted at runtime):
  - softmax over a size-1 axis == 1, so MHA(x_q, x_kv) == x_kv @ wa with
    wa = wv.T @ ow.T  (one [512,512] matmul per stream per layer).
  - LayerNorm == center + scale.  Centering is folded into the weights on
    host (W' = W @ C with C = I - 11^T/D): every pre-LN activation z is then
    exactly zero-mean, so LN(z) = z * rsqrt(mean(z^2)+eps).
  - The attention-side LNs are skipped entirely: relu is positively
    homogeneous, so the per-row scale rstd1 of LN1 cancels inside the next
    LN (error only via eps, ~1e-7 relative).  Only the 2 ffn-side LNs per
    layer are computed.

Device layout (per core, data-parallel over candidate rows):
  activations are FEATURE-MAJOR: [128 feature-partitions, 4 chunks x R rows]
  with R=512 rows per block.  Matmuls use weight chunks as lhsT (stationary)
  and activations as rhs (moving) -> outputs stay feature-major; NO on-device
  transposes anywhere.  Residual adds ride the PSUM accumulation group as
  identity-weight matmuls.  Row-wise sum(z^2) via ones-vector matmuls,
  rstd broadcast across partitions via a K=1 ones matmul.  Layer-0's
  candidate-mean term is restored by a rank-1 matmul (m_c x vrow).
  Sigmoid is applied on host.
"""

import sys

import numpy as np
import ml_dtypes

N = 131072
D = 512
HID = 256
L = 2
P = 128
R = 512          # rows per block (free dim; one PSUM bank per m-chunk)
NCORES = 8
EPS = 1e-5

BF16 = ml_dtypes.bfloat16

_cache: dict = {}


def _chunk_lhsT(w: np.ndarray) -> np.ndarray:
    """[K, M] -> [128, (K//128)*(M//128)*128]; chunk (k,m) = w[k*128:.., m*128:..]
    laid out at columns (k*(M//128)+m)*128."""
    K, M = w.shape
    nk, nm = K // P, M // P
    out = np.empty((P, nk * nm * P), np.float64)
    for k in range(nk):
        for m in range(nm):
            out[:, (k * nm + m) * P : (k * nm + m + 1) * P] = w[
                k * P : (k + 1) * P, m * P : (m + 1) * P
            ]
    return out


def _prep_host(inputs):
    """Fold weights on host (fp64), center, cast bf16, chunk for SBUF."""
    f8 = np.float64
    assert np.all(np.asarray(inputs["ln_w"]) == 1.0), "kernel assumes ln_w == 1"
    assert not np.any(np.asarray(inputs["ln_b"])), "kernel assumes ln_b == 0"
    for k in ("attn_in_b", "attn_out_b", "ffn_b1", "ffn_b2", "head_b1", "head_b2"):
        assert not np.any(np.asarray(inputs[k])), f"kernel assumes {k} == 0"

    C = np.eye(D, dtype=f8) - np.full((D, D), 1.0 / D, dtype=f8)
    arrs = {}
    for i in range(L):
        wv = np.asarray(inputs["attn_in_w"])[i][2 * D :].astype(f8)  # [D, D]
        ow = np.asarray(inputs["attn_out_w"])[i].astype(f8)          # [D, D]
        wa = (wv.T @ ow.T) @ C                                       # [D, D] centered out
        arrs[f"wa{i}"] = _chunk_lhsT(wa).astype(BF16)                # [128, 16*128]
        if i == 0:
            arrs["vrow"] = wa.sum(axis=0).reshape(1, D).astype(BF16)  # [1, 512]
        w1 = np.asarray(inputs["ffn_w1"])[i].T.astype(f8)            # [D, HID]
        arrs[f"w1_{i}"] = _chunk_lhsT(w1).astype(BF16)               # [128, 8*128]
        w2 = np.asarray(inputs["ffn_w2"])[i].T.astype(f8) @ C        # [HID, D] centered
        arrs[f"w2_{i}"] = _chunk_lhsT(w2).astype(BF16)               # [128, 8*128]
    arrs["h1"] = _chunk_lhsT(np.asarray(inputs["head_w1"]).T.astype(f8)).astype(BF16)
    arrs["h2"] = np.asarray(inputs["head_w2"]).T.astype(f8).astype(BF16)  # [256,1]->[128,2]
    arrs["h2"] = np.ascontiguousarray(
        np.asarray(inputs["head_w2"]).astype(f8).reshape(2, P).T
    ).astype(BF16)                                                   # [128, 2], col k
    q0 = np.asarray(inputs["query_embedding"]).astype(f8).reshape(D)
    q0c = q0 - q0.mean()
    # q0rep: chunk j at cols [j*R:(j+1)*R], replicated across R rows
    q0rep = np.repeat(q0c.reshape(4, P).T[:, :, None], R, axis=2).reshape(P, 4 * R)
    arrs["q0rep"] = np.ascontiguousarray(q0rep).astype(BF16)
    arrs["identb"] = np.eye(P, dtype=np.float32).astype(BF16)
    arrs["ones128"] = np.ones((P, 1), np.float32).astype(BF16)
    arrs["onesbc"] = np.ones((1, P), np.float32).astype(BF16)
    return arrs


def _prep_cand(inputs):
    """Center candidates, return feature-major chunked bf16 + row means."""
    cand = np.asarray(inputs["candidate_embeddings"]).astype(np.float32)  # [N, D]
    m_c = cand.mean(axis=1)                                               # [N]
    cen = cand - m_c[:, None]
    # per core: [Nc, D] -> [128, nblocks*4*R] with col ((b*4)+j)*R + r
    return cen, m_c


def _pack_core(cen_core: np.ndarray) -> np.ndarray:
    nc_rows = cen_core.shape[0]
    nb = nc_rows // R
    # [nb, R, 4, 128] -> [128, nb, 4, R]
    x = cen_core.reshape(nb, R, 4, P).transpose(3, 0, 2, 1)
    return np.ascontiguousarray(x.reshape(P, nb * 4 * R)).astype(BF16)


def _build_program(rows_per_core: int, repeat: int = 1):
    import concourse.bass as bass
    import concourse.mybir as mybir
    import concourse.tile as tile
    from concourse import bacc
    from concourse.bass import ts

    dt = mybir.dt
    alu = mybir.AluOpType
    act_fn = mybir.ActivationFunctionType
    nblocks = rows_per_core // R
    assert rows_per_core % R == 0

    nc = bacc.Bacc(
        "TRN2", target_bir_lowering=False, debug=False, num_devices=NCORES
    )

    cand = nc.dram_tensor(
        "cand", [P, nblocks * 4 * R], dt.bfloat16, kind="ExternalInput"
    )
    mc = nc.dram_tensor("mc", [1, rows_per_core], dt.bfloat16, kind="ExternalInput")
    dr = {}
    for i in range(L):
        dr[f"wa{i}"] = nc.dram_tensor(f"wa{i}", [P, 16 * P], dt.bfloat16, kind="ExternalInput")
        dr[f"w1_{i}"] = nc.dram_tensor(f"w1_{i}", [P, 8 * P], dt.bfloat16, kind="ExternalInput")
        dr[f"w2_{i}"] = nc.dram_tensor(f"w2_{i}", [P, 8 * P], dt.bfloat16, kind="ExternalInput")
    dr["h1"] = nc.dram_tensor("h1", [P, 16 * P], dt.bfloat16, kind="ExternalInput")
    dr["h2"] = nc.dram_tensor("h2", [P, 2], dt.bfloat16, kind="ExternalInput")
    dr["q0rep"] = nc.dram_tensor("q0rep", [P, 4 * R], dt.bfloat16, kind="ExternalInput")
    dr["vrow"] = nc.dram_tensor("vrow", [1, D], dt.bfloat16, kind="ExternalInput")
    dr["ones128"] = nc.dram_tensor("ones128", [P, 1], dt.bfloat16, kind="ExternalInput")
    dr["onesbc"] = nc.dram_tensor("onesbc", [1, P], dt.bfloat16, kind="ExternalInput")
    scores = nc.dram_tensor("scores", [1, rows_per_core], dt.float32, kind="ExternalOutput")

    from contextlib import ExitStack

    with tile.TileContext(nc) as tc, ExitStack() as ctx:
        const = ctx.enter_context(tc.tile_pool(name="const", bufs=1))

        def load_const(name, shape, dtype):
            # scalar-engine DMA queue: const loads run in parallel with the
            # sync queue's first cin block loads (kills the startup PE stall)
            t = const.tile(shape, dtype, tag=f"const_{name}", name=f"c_{name}")
            nc.scalar.dma_start(t[:], dr[name].ap())
            return t

        wsb = []
        for i in range(L):
            wsb.append(
                (
                    load_const(f"wa{i}", [P, 16 * P], dt.bfloat16),
                    load_const(f"w1_{i}", [P, 8 * P], dt.bfloat16),
                    load_const(f"w2_{i}", [P, 8 * P], dt.bfloat16),
                )
            )
        h1sb = load_const("h1", [P, 16 * P], dt.bfloat16)
        h2sb = load_const("h2", [P, 2], dt.bfloat16)
        q0sb = load_const("q0rep", [P, 4 * R], dt.bfloat16)
        vrow = load_const("vrow", [1, D], dt.bfloat16)
        ones128 = load_const("ones128", [P, 1], dt.bfloat16)
        onesbc = load_const("onesbc", [1, P], dt.bfloat16)
        mcsb = const.tile([1, rows_per_core], dt.bfloat16, tag="const_mc", name="c_mc")
        nc.scalar.dma_start(mcsb[:], mc.ap())
        eps_t = const.tile([1, 1], dt.float32, tag="const_eps", name="c_eps")
        nc.gpsimd.memset(eps_t[:], float(EPS))

        pin = ctx.enter_context(tc.tile_pool(name="pin", bufs=4))
        zp = ctx.enter_context(tc.tile_pool(name="zp", bufs=4))
        hp = ctx.enter_context(tc.tile_pool(name="hp", bufs=4))
        up = ctx.enter_context(tc.tile_pool(name="up", bufs=3))
        zzp = ctx.enter_context(tc.tile_pool(name="zzp", bufs=3))
        ap_ = ctx.enter_context(tc.tile_pool(name="ap", bufs=10))
        stp = ctx.enter_context(tc.tile_pool(name="stp", bufs=4))
        rbp = ctx.enter_context(tc.tile_pool(name="rbp", bufs=3))
        fin = ctx.enter_context(tc.tile_pool(name="fin", bufs=3))
        pY = ctx.enter_context(tc.tile_pool(name="pY", bufs=2, space="PSUM"))
        pH = ctx.enter_context(tc.tile_pool(name="pH", bufs=2, space="PSUM"))
        pS = ctx.enter_context(tc.tile_pool(name="pS", bufs=1, space="PSUM"))
        pB = ctx.enter_context(tc.tile_pool(name="pB", bufs=1, space="PSUM"))

        def attn_block(wa, x_sb, resid_sb, rank1_rhs):
            """z[m] = sum_k wa[k,m].T x[k] + resid[m] (+ vrow[m] x m_c)."""
            z = zp.tile([P, 4 * R], dt.bfloat16, name="z")
            for g in range(2):
                y = pY.tile([P, 2 * R], dt.float32, tag="y", name="ypsum")
                for mm_ in range(2):
                    m = 2 * g + mm_
                    for k in range(4):
                        nc.tensor.matmul(
                            y[:, ts(mm_, R)],
                            wa[:, ts(k * 4 + m, P)],
                            x_sb[:, ts(k, R)],
                            start=(k == 0),
                            stop=(k == 3 and rank1_rhs is None),
                        )
                    if rank1_rhs is not None:
                        nc.tensor.matmul(
                            y[:, ts(mm_, R)], vrow[0:1, ts(m, P)], rank1_rhs,
                            start=False, stop=True,
                        )
                nc.vector.scalar_tensor_tensor(
                    out=z[:, ts(g, 2 * R)], in0=y[:, :], scalar=1.0,
                    in1=resid_sb[:, ts(g, 2 * R)], op0=alu.bypass, op1=alu.add,
                )
            return z

        def ffn_ln_block(w1, w2, z_sb):
            """a = LN(z + relu(z@w1)@w2'), returned bf16 feature-major."""
            h = hp.tile([P, 2 * R], dt.bfloat16, name="h")
            for m in range(2):
                hps = pH.tile([P, R], dt.float32, tag="h", name="hpsum")
                for k in range(4):
                    nc.tensor.matmul(
                        hps[:, :],
                        w1[:, ts(k * 2 + m, P)],
                        z_sb[:, ts(k, R)],
                        start=(k == 0),
                        stop=(k == 3),
                    )
                nc.scalar.activation(h[:, ts(m, R)], hps[:, :], act_fn.Relu)
            u = up.tile([P, 4 * R], dt.bfloat16, name="u")
            zz = zzp.tile([P, 4 * R], dt.bfloat16, name="zz")
            for g in range(2):
                u_ps = pY.tile([P, 2 * R], dt.float32, tag="y", name="upsum")
                for mm_ in range(2):
                    m = 2 * g + mm_
                    for k in range(2):
                        nc.tensor.matmul(
                            u_ps[:, ts(mm_, R)],
                            w2[:, ts(k * 4 + m, P)],
                            h[:, ts(k, R)],
                            start=(k == 0),
                            stop=(k == 1),
                        )
                nc.vector.scalar_tensor_tensor(
                    out=u[:, ts(g, 2 * R)], in0=u_ps[:, :], scalar=1.0,
                    in1=z_sb[:, ts(g, 2 * R)], op0=alu.bypass, op1=alu.add,
                )
                nc.scalar.activation(zz[:, ts(g, 2 * R)], u[:, ts(g, 2 * R)],
                                     act_fn.Square)
            sq = pS.tile([1, R], dt.float32, tag="sq", name="sqpsum")
            for k in range(4):
                nc.tensor.matmul(
                    sq[0:1, :], ones128[:], zz[:, ts(k, R)],
                    start=(k == 0), stop=(k == 3),
                )
            st = stp.tile([1, R], dt.float32, tag="std", name="std")
            nc.scalar.activation(st[0:1, :], sq[0:1, :], act_fn.Sqrt,
                                 bias=eps_t[0:1, :], scale=1.0 / D)
            rstd = stp.tile([1, R], dt.bfloat16, tag="rstd", name="rstd")
            with nc.allow_low_precision(reason="per-row rstd scale; bf16 ok at 2e-2 tol"):
                nc.vector.reciprocal(out=rstd[0:1, :], in_=st[0:1, :])
            rb = pB.tile([P, R], dt.float32, tag="rb", name="rbpsum")
            nc.tensor.matmul(rb[:, :], onesbc[0:1, :], rstd[0:1, :],
                             start=True, stop=True)
            rbs = rbp.tile([P, 2 * R], dt.bfloat16, name="rbs")
            nc.scalar.copy(rbs[:, 0:R], rb[:, :])
            nc.scalar.copy(rbs[:, R : 2 * R], rb[:, :])
            a = ap_.tile([P, 4 * R], dt.bfloat16, name="a")
            for g in range(2):
                nc.vector.tensor_tensor(
                    out=a[:, ts(g, 2 * R)], in0=u[:, ts(g, 2 * R)],
                    in1=rbs[:, :], op=alu.mult,
                )
            return a

        def head_block(q_state, c_state, b):
            hh = hp.tile([P, 2 * R], dt.bfloat16, name="hh")
            for m in range(2):
                hh_ps = pH.tile([P, R], dt.float32, tag="h", name="hhpsum")
                for k in range(8):
                    src = q_state if k < 4 else c_state
                    nc.tensor.matmul(
                        hh_ps[:, :],
                        h1sb[:, ts(k * 2 + m, P)],
                        src[:, ts(k % 4, R)],
                        start=(k == 0),
                        stop=(k == 7),
                    )
                nc.scalar.activation(hh[:, ts(m, R)], hh_ps[:, :], act_fn.Relu)
            lg = pS.tile([1, R], dt.float32, tag="sq", name="lgpsum")
            for k in range(2):
                nc.tensor.matmul(
                    lg[0:1, :], h2sb[:, k : k + 1], hh[:, ts(k, R)],
                    start=(k == 0), stop=(k == 1),
                )
            out_t = fin.tile([1, R], dt.float32, name="out_t")
            nc.scalar.copy(out_t[0:1, :], lg[0:1, :])
            nc.sync.dma_start(scores.ap()[0:1, b * R : (b + 1) * R], out_t[0:1, :])

        # two-block software pipeline: stages of blocks (b, b+1) interleave so
        # each engine's in-order stream alternates between independent chains.
        assert nblocks % 2 == 0
        for pb_rep in range(repeat * (nblocks // 2)):
            pb = pb_rep % (nblocks // 2)
            pair = (2 * pb, 2 * pb + 1)
            cins = {}
            for b in pair:
                cin = pin.tile([P, 4 * R], dt.bfloat16, name="cin")
                nc.sync.dma_start(cin[:], cand.ap()[:, b * 4 * R : (b + 1) * 4 * R])
                cins[b] = cin
            qs = {b: q0sb for b in pair}
            cs = {b: cins[b] for b in pair}
            zt = {}
            for i in range(L):
                wa, w1, w2 = wsb[i]
                for b in pair:
                    zt[b] = attn_block(wa, cs[b], qs[b],
                                       mcsb[0:1, b * R : (b + 1) * R] if i == 0 else None)
                for b in pair:
                    qs[b] = ffn_ln_block(w1, w2, zt[b])
                for b in pair:
                    zt[b] = attn_block(wa, qs[b], cs[b], None)
                for b in pair:
                    cs[b] = ffn_ln_block(w1, w2, zt[b])
            for b in pair:
                head_block(qs[b], cs[b], b)

    nc.compile()
    return nc


def _get_program(rows_per_core: int):
    if rows_per_core not in _cache:
        _cache[rows_per_core] = _build_program(rows_per_core)
    return _cache[rows_per_core]


def build_in_maps(inputs):
    """Host prep -> (nc, in_maps) for run_bass_kernel_spmd."""
    arrs = _prep_host(inputs)
    cen, m_c = _prep_cand(inputs)
    n = cen.shape[0]
    rows = n // NCORES
    nc = _get_program(rows)
    in_maps = []
    for c in range(NCORES):
        m = dict(arrs)
        m["cand"] = _pack_core(cen[c * rows : (c + 1) * rows])
        m["mc"] = np.ascontiguousarray(
            m_c[c * rows : (c + 1) * rows].reshape(1, rows)
        ).astype(BF16)
        in_maps.append(m)
    return nc, in_maps


def kernel(**inputs) -> np.ndarray:
    from concourse.bass_utils import run_bass_kernel_spmd

    nc, in_maps = build_in_maps(inputs)
    n = np.asarray(inputs["candidate_embeddings"]).shape[0]
    rows = n // NCORES
    res = run_bass_kernel_spmd(nc, in_maps, list(range(NCORES)))
    logits = np.concatenate(
        [res.results[c]["scores"].reshape(rows) for c in range(NCORES)], axis=0
    ).astype(np.float64)
    out = 1.0 / (1.0 + np.exp(-logits))
    return out.reshape(n, 1).astype(np.float32)


if __name__ == "__main__":
    rows = int(sys.argv[1]) if len(sys.argv) > 1 else 2048
    nc = _build_program(rows)
    print("built ok:", rows)
